# revision 5
# baseline (speedup 1.0000x reference)
"""Chamfer distance kernel for Trainium2 (8 NeuronCores, data-parallel over batch).

Math: d[n,m] = |a_n|^2 + |b_m|^2 - 2 a_n . b_m, computed as a K=5 augmented
matmul: aug1 = [x,y,z,|a|^2,1], aug2 = [-2x,-2y,-2z,1,|b|^2], so
aug1[:,n] . aug2[:,m] = d[n,m].  dist1 = min over m (after relu clamp),
dist2 = min over n.

Per core (one batch):
  - PE: fp32r matmuls [5,128]x[5,512] -> PSUM fp32, grouped 4 banks at a time
  - ACT: cast PSUM fp32 [128,2048] -> SBUF fp16 (staged)
  - DVE tensor_scalar (4x mode): relu clamp in place + min-reduce over free
    -> dist1 partials
  - DVE tensor_tensor min: accumulate staged into acc2 [128, M] fp16 (dist2
    partial per m over the 128-lane n-residual)
  - tail: PE transpose acc2 in 128x128 blocks -> PSUM, TS min-reduce -> dist2
"""

import sys

sys.path.insert(0, "/opt/trn_rl_repo")

import numpy as np

import concourse.bass as bass  # noqa: F401  (bass types used via bacc/tile)
import concourse.mybir as mybir
import concourse.tile as tile
from concourse import bacc, bass_isa
from concourse.bass_utils import run_bass_kernel_spmd
from concourse.masks import make_identity

B, N, M = 8, 8192, 8192
P = 128           # partitions / n-chunk size
MT = 512          # m tile (one PSUM bank of fp32)
G = 4             # m tiles per PSUM group
GF = G * MT       # 2048 free elems per group
NCH = N // P      # 64 n-chunks
NG = M // GF      # 4 groups of m
F32 = mybir.dt.float32
F32R = mybir.dt.float32r
F16 = mybir.dt.float16
BF16 = mybir.dt.bfloat16

_cache = {}


def build(precision="fp32r", repeat=1, opt=5):
    # opt>=1: tail uses transpose-merged tensor_reduce
    # opt==4: full-row staging [128,8192]: one TS (dist1 direct) + one TT
    #         per n-chunk; ~5% of casts shifted from ACT to DVE for balance
    # (opt 2/3 = GpSimd offload variants: rejected by this walrus, unused)
    if opt >= 6:
        return build_v2()
    key = (precision, repeat, opt)
    if key in _cache:
        return _cache[key]
    nc = bacc.Bacc()
    if precision == "fp32r":
        aug1 = nc.declare_dram_parameter("aug1", [5, N], F32, isOutput=False)
        aug2 = nc.declare_dram_parameter("aug2", [5, M], F32, isOutput=False)
    else:  # bf16x3: host-split hi/lo
        a1h = nc.declare_dram_parameter("a1h", [5, N], BF16, isOutput=False)
        a1l = nc.declare_dram_parameter("a1l", [5, N], BF16, isOutput=False)
        a2h = nc.declare_dram_parameter("a2h", [5, M], BF16, isOutput=False)
        a2l = nc.declare_dram_parameter("a2l", [5, M], BF16, isOutput=False)
    out1 = nc.declare_dram_parameter("out1", [P, NCH], F32, isOutput=True)
    out2 = nc.declare_dram_parameter("out2", [P, NCH], F32, isOutput=True)

    with tile.TileContext(nc) as tc:
        with tc.tile_pool(name="const", bufs=1) as const, \
             tc.tile_pool(name="stage", bufs=(2 if opt == 4 else 3)) as stage, \
             tc.tile_pool(name="small", bufs=4) as small, \
             tc.tile_pool(name="psum", bufs=2, space="PSUM") as psum:

            if precision == "fp32r":
                a1_sb = const.tile([5, N], F32)
                a2_sb = const.tile([5, M], F32)
                nc.sync.dma_start(out=a1_sb[:], in_=aug1[:])
                nc.sync.dma_start(out=a2_sb[:], in_=aug2[:])
                a1r = const.tile([5, N], F32R)
                a2r = const.tile([5, M], F32R)
                nc.vector.tensor_copy(a2r[:, 0:GF], a2_sb[:, 0:GF])
                nc.vector.tensor_copy(a1r[:, 0:P], a1_sb[:, 0:P])
                nc.vector.tensor_copy(a2r[:, GF:M], a2_sb[:, GF:M])
                nc.vector.tensor_copy(a1r[:, P:N], a1_sb[:, P:N])
                lhs_list = [a1r]
                rhs_list = [a2r]
            else:
                a1h_sb = const.tile([5, N], BF16)
                a1l_sb = const.tile([5, N], BF16)
                a2h_sb = const.tile([5, M], BF16)
                a2l_sb = const.tile([5, M], BF16)
                nc.sync.dma_start(out=a1h_sb[:], in_=a1h[:])
                nc.sync.dma_start(out=a1l_sb[:], in_=a1l[:])
                nc.sync.dma_start(out=a2h_sb[:], in_=a2h[:])
                nc.sync.dma_start(out=a2l_sb[:], in_=a2l[:])
                # hi*hi + hi*lo + lo*hi (lo*lo dropped, ~2^-16 relative)
                lhs_list = [a1h_sb, a1h_sb, a1l_sb]
                rhs_list = [a2h_sb, a2l_sb, a2h_sb]

            acc2 = const.tile([P, M], F16)
            nc.vector.memset(acc2[:], float("inf"))
            dist1_sb = const.tile([P, NCH], F32)
            dist2_sb = const.tile([P, NCH], F32)
            ident = const.tile([P, P], F16)
            make_identity(nc, ident[:])

            npass = len(lhs_list)
            for _rep in range(repeat):
              for nci in range(NCH):
                if opt >= 4:
                    # half-row (opt 5) or full-row (opt 4) staging
                    nh = 2 if opt >= 5 else 1
                    HF = M // nh
                    part1 = None
                    if nh > 1:
                        part1 = small.tile([P, nh], F32, tag="part1")
                    for h in range(nh):
                        sth = stage.tile([P, HF], F16, tag="st")
                        for g2 in range(NG // nh):
                            g = h * (NG // nh) + g2
                            ps = psum.tile([P, GF], F32, tag="ps")
                            for q in range(G):
                                mc = g * G + q
                                for ip in range(npass):
                                    nc.tensor.matmul(
                                        out=ps[:, q * MT:(q + 1) * MT],
                                        lhsT=lhs_list[ip][:, nci * P:(nci + 1) * P],
                                        rhs=rhs_list[ip][:, mc * MT:(mc + 1) * MT],
                                        start=(ip == 0),
                                        stop=(ip == npass - 1),
                                    )
                            if g == 1 and nci % 9 == 0:
                                nc.vector.tensor_copy(sth[:, g2 * GF:(g2 + 1) * GF], ps[:])
                            else:
                                nc.scalar.copy(out=sth[:, g2 * GF:(g2 + 1) * GF], in_=ps[:])
                        acc1_dst = dist1_sb[:, nci:nci + 1] if nh == 1 else part1[:, h:h + 1]
                        nc.vector.tensor_scalar(
                            out=sth[:], in0=sth[:], scalar1=0.0, scalar2=None,
                            op0=mybir.AluOpType.max, op1=mybir.AluOpType.min,
                            accum_out=acc1_dst,
                        )
                        nc.vector.tensor_tensor(
                            out=acc2[:, h * HF:(h + 1) * HF],
                            in0=acc2[:, h * HF:(h + 1) * HF], in1=sth[:],
                            op=mybir.AluOpType.min,
                        )
                    if nh > 1:
                        nc.vector.tensor_reduce(
                            out=dist1_sb[:, nci:nci + 1], in_=part1[:],
                            axis=mybir.AxisListType.X, op=mybir.AluOpType.min,
                        )
                    continue
                part1 = small.tile([P, NG], F32)
                for g in range(NG):
                    ps = psum.tile([P, GF], F32, tag="ps")
                    for q in range(G):
                        mc = g * G + q
                        for ip in range(npass):
                            nc.tensor.matmul(
                                out=ps[:, q * MT:(q + 1) * MT],
                                lhsT=lhs_list[ip][:, nci * P:(nci + 1) * P],
                                rhs=rhs_list[ip][:, mc * MT:(mc + 1) * MT],
                                start=(ip == 0),
                                stop=(ip == npass - 1),
                            )
                    st = stage.tile([P, GF], F16, tag="st")
                    nc.scalar.copy(out=st[:], in_=ps[:])
                    nc.vector.tensor_scalar(
                        out=st[:], in0=st[:], scalar1=0.0, scalar2=None,
                        op0=mybir.AluOpType.max, op1=mybir.AluOpType.min,
                        accum_out=part1[:, g:g + 1],
                    )
                    nc.vector.tensor_tensor(
                        out=acc2[:, g * GF:(g + 1) * GF],
                        in0=acc2[:, g * GF:(g + 1) * GF],
                        in1=st[:], op=mybir.AluOpType.min,
                    )
                nc.vector.tensor_reduce(
                    out=dist1_sb[:, nci:nci + 1], in_=part1[:],
                    axis=mybir.AxisListType.X, op=mybir.AluOpType.min,
                )

              # dist2 tail: transpose acc2 128x128 blocks, min-reduce over n
              if opt >= 1:
                for t4 in range(NCH // 4):
                    pst = psum.tile([P, 4, P], F16, tag="ps")
                    for q in range(4):
                        nc.tensor.transpose(
                            out=pst[:, q, :],
                            in_=acc2[:, (t4 * 4 + q) * P:(t4 * 4 + q + 1) * P],
                            identity=ident[:],
                        )
                    nc.vector.tensor_reduce(
                        out=dist2_sb[:, t4 * 4:t4 * 4 + 4], in_=pst[:],
                        axis=mybir.AxisListType.X, op=mybir.AluOpType.min,
                    )
                # final relu clamp for dist2 (dist1 clamps in the TS)
                nc.vector.tensor_scalar(
                    out=dist2_sb[:], in0=dist2_sb[:], scalar1=0.0, scalar2=None,
                    op0=mybir.AluOpType.max,
                )
              else:
                for t in range(NCH):
                    pst = psum.tile([P, P], F16, tag="ps")
                    nc.tensor.transpose(
                        out=pst[:], in_=acc2[:, t * P:(t + 1) * P], identity=ident[:]
                    )
                    scr = small.tile([P, P], F16)
                    nc.vector.tensor_scalar(
                        out=scr[:], in0=pst[:], scalar1=0.0, scalar2=None,
                        op0=mybir.AluOpType.max, op1=mybir.AluOpType.min,
                        accum_out=dist2_sb[:, t:t + 1],
                    )

            nc.sync.dma_start(out=out1[:], in_=dist1_sb[:])
            nc.sync.dma_start(out=out2[:], in_=dist2_sb[:])

    nc.finalize()
    _cache[key] = nc
    return nc


def build_v2(q=4, stage_bufs=3):
    """Negated-max formulation, no transpose tail.

    aug2n is pre-negated on the host so the matmul produces s = -d.
    Per chunk: ACT casts PSUM->st fp16 (3 or 4 of 4 groups); on every q-th
    chunk the DVE drains the last group via a fused TS (cast + clamp +
    max-accum -> dist1 partial).  One full-span TS (4x mode) gives the dist1
    partial of the ACT-cast span; one TT max merges st into acc2.  Tail:
    GpSimd partition_all_reduce(max) over acc2 quarters (in-place), DMA row 0.
    dist1/dist2 are negated+relu'd on the host.
    """
    key = ("v2", q, stage_bufs)
    if key in _cache:
        return _cache[key]
    nc = bacc.Bacc()
    aug1 = nc.declare_dram_parameter("aug1", [5, N], F32, isOutput=False)
    aug2n = nc.declare_dram_parameter("aug2n", [5, M], F32, isOutput=False)
    out1 = nc.declare_dram_parameter("out1", [P, NCH], F32, isOutput=True)
    out2 = nc.declare_dram_parameter("out2", [1, M], F16, isOutput=True)

    with tile.TileContext(nc) as tc:
        with tc.tile_pool(name="const", bufs=1) as const, \
             tc.tile_pool(name="stage", bufs=stage_bufs) as stage, \
             tc.tile_pool(name="psum", bufs=2, space="PSUM") as psum:

            l1 = const.tile([5, N], F32)
            l2 = const.tile([5, M], F32)
            nc.sync.dma_start(out=l1[:], in_=aug1[:])
            nc.sync.dma_start(out=l2[:], in_=aug2n[:])
            a1r = const.tile([5, N], F32R)
            a2r = const.tile([5, M], F32R)
            # get the first chunk's operands ready fast (DVE), bulk on ACT/DVE
            nc.vector.tensor_copy(a2r[:, 0:GF], l2[:, 0:GF])
            nc.vector.tensor_copy(a1r[:, 0:P], l1[:, 0:P])
            nc.scalar.copy(out=a1r[:, P:N], in_=l1[:, P:N])
            nc.vector.tensor_copy(a2r[:, GF:M], l2[:, GF:M])

            acc2 = const.tile([P, M], F16)
            nc.gpsimd.memset(acc2[:], float("-inf"))
            parts0 = const.tile([P, NCH], F32)
            parts1 = const.tile([P, NCH], F32)
            nc.gpsimd.memset(parts1[:], -1e30)
            d1row = const.tile([P, NCH], F32)

            for nci in range(NCH):
                dve_drain = (nci % q == q - 1)
                ng_act = NG - 1 if dve_drain else NG
                st = stage.tile([P, M], F16, tag="st")
                for g in range(NG):
                    ps = psum.tile([P, GF], F32, tag="ps")
                    for mt in range(G):
                        mc = g * G + mt
                        nc.tensor.matmul(
                            out=ps[:, mt * MT:(mt + 1) * MT],
                            lhsT=a1r[:, nci * P:(nci + 1) * P],
                            rhs=a2r[:, mc * MT:(mc + 1) * MT],
                            start=True, stop=True,
                        )
                    if g < ng_act:
                        nc.scalar.copy(out=st[:, g * GF:(g + 1) * GF], in_=ps[:])
                    else:
                        nc.vector.tensor_scalar(
                            out=st[:, g * GF:(g + 1) * GF], in0=ps[:],
                            scalar1=0.0, scalar2=None,
                            op0=mybir.AluOpType.min, op1=mybir.AluOpType.max,
                            accum_out=parts1[:, nci:nci + 1],
                        )
                nc.vector.tensor_scalar(
                    out=st[:, 0:ng_act * GF], in0=st[:, 0:ng_act * GF],
                    scalar1=0.0, scalar2=None,
                    op0=mybir.AluOpType.min, op1=mybir.AluOpType.max,
                    accum_out=parts0[:, nci:nci + 1],
                )
                if nci < NCH - 1:
                    nc.vector.tensor_tensor(
                        out=acc2[:], in0=acc2[:], in1=st[:],
                        op=mybir.AluOpType.max,
                    )
                else:
                    # final chunk: merge + partition-reduce + store by quarters
                    QF = M // 4
                    for qt in range(4):
                        sl = slice(qt * QF, (qt + 1) * QF)
                        nc.vector.tensor_tensor(
                            out=acc2[:, sl], in0=acc2[:, sl], in1=st[:, sl],
                            op=mybir.AluOpType.max,
                        )
                        nc.gpsimd.partition_all_reduce(
                            acc2[:, sl], acc2[:, sl], channels=P,
                            reduce_op=bass_isa.ReduceOp.max,
                        )
                        nc.sync.dma_start(out=out2[:, sl], in_=acc2[0:1, sl])

            nc.vector.tensor_tensor(
                out=d1row[:], in0=parts0[:], in1=parts1[:],
                op=mybir.AluOpType.max,
            )
            nc.sync.dma_start(out=out1[:], in_=d1row[:])

    nc.finalize()
    _cache[key] = nc
    return nc


def _augment(p1, p2):
    """p1 [N,3], p2 [M,3] -> aug1 [5,N], aug2 [5,M] fp32 (C-contiguous)."""
    sq1 = (p1 * p1).sum(1)
    sq2 = (p2 * p2).sum(1)
    aug1 = np.empty((5, p1.shape[0]), np.float32)
    aug1[0:3] = p1.T
    aug1[3] = sq1
    aug1[4] = 1.0
    aug2 = np.empty((5, p2.shape[0]), np.float32)
    aug2[0:3] = -2.0 * p2.T
    aug2[3] = 1.0
    aug2[4] = sq2
    return aug1, aug2


def make_in_maps(input1, input2, precision="fp32r"):
    import ml_dtypes
    in_maps = []
    for b in range(B):
        p1 = np.ascontiguousarray(np.asarray(input1[b], np.float32))
        p2 = np.ascontiguousarray(np.asarray(input2[b], np.float32))
        aug1, aug2 = _augment(p1, p2)
        if precision == "fp32r":
            in_maps.append({"aug1": aug1, "aug2": aug2})
        else:
            a1h = aug1.astype(ml_dtypes.bfloat16)
            a1l = (aug1 - a1h.astype(np.float32)).astype(ml_dtypes.bfloat16)
            a2h = aug2.astype(ml_dtypes.bfloat16)
            a2l = (aug2 - a2h.astype(np.float32)).astype(ml_dtypes.bfloat16)
            in_maps.append({"a1h": a1h, "a1l": a1l, "a2h": a2h, "a2l": a2l})
    return in_maps


def make_in_maps_v2(input1, input2):
    in_maps = []
    for b in range(B):
        p1 = np.ascontiguousarray(np.asarray(input1[b], np.float32))
        p2 = np.ascontiguousarray(np.asarray(input2[b], np.float32))
        aug1, aug2 = _augment(p1, p2)
        in_maps.append({"aug1": aug1, "aug2n": np.ascontiguousarray(-aug2)})
    return in_maps


def run_v2(input1, input2, q=4, stage_bufs=3, trace=False):
    nc = build_v2(q=q, stage_bufs=stage_bufs)
    in_maps = make_in_maps_v2(input1, input2)
    res = run_bass_kernel_spmd(nc, in_maps, list(range(B)), trace=trace)
    # out1 [P, NCH]: lane p, chunk c -> n = c*128+p; value = max_m(-d) (clamped)
    dist1 = np.stack([
        np.maximum(-res.results[b]["out1"].T.reshape(N), 0.0) for b in range(B)
    ])
    dist2 = np.stack([
        np.maximum(-np.asarray(res.results[b]["out2"], np.float32).reshape(M), 0.0)
        for b in range(B)
    ])
    return (dist1.astype(np.float32), dist2.astype(np.float32)), res


def run(input1, input2, precision="fp32r", trace=False, opt=5, repeat=1):
    if opt >= 6:
        return run_v2(input1, input2, trace=trace)
    nc = build(precision, repeat=repeat, opt=opt)
    in_maps = make_in_maps(input1, input2, precision)
    res = run_bass_kernel_spmd(nc, in_maps, list(range(B)), trace=trace)
    dist1 = np.stack([res.results[b]["out1"].T.reshape(N) for b in range(B)])
    dist2 = np.stack([res.results[b]["out2"].T.reshape(N) for b in range(B)])
    return (dist1.astype(np.float32), dist2.astype(np.float32)), res


def kernel(input1, input2):
    (dist1, dist2), _ = run(input1, input2, precision="fp32r", trace=False, opt=6)
    return (dist1, dist2)



# revision 34
# speedup vs baseline: 1.0379x; 1.0379x over previous
"""Chamfer distance kernel for Trainium2 (8 NeuronCores, data-parallel over batch).

Math: d[n,m] = |a_n|^2 + |b_m|^2 - 2 a_n . b_m, computed as a K=5 augmented
matmul: aug1 = [x,y,z,|a|^2,1], aug2 = [-2x,-2y,-2z,1,|b|^2], so
aug1[:,n] . aug2[:,m] = d[n,m].  dist1 = min over m (after relu clamp),
dist2 = min over n.

Per core (one batch):
  - PE: fp32r matmuls [5,128]x[5,512] -> PSUM fp32, grouped 4 banks at a time
  - ACT: cast PSUM fp32 [128,2048] -> SBUF fp16 (staged)
  - DVE tensor_scalar (4x mode): relu clamp in place + min-reduce over free
    -> dist1 partials
  - DVE tensor_tensor min: accumulate staged into acc2 [128, M] fp16 (dist2
    partial per m over the 128-lane n-residual)
  - tail: PE transpose acc2 in 128x128 blocks -> PSUM, TS min-reduce -> dist2
"""

import sys

sys.path.insert(0, "/opt/trn_rl_repo")

import numpy as np

import concourse.bass as bass  # noqa: F401  (bass types used via bacc/tile)
import concourse.mybir as mybir
import concourse.tile as tile
from concourse import bacc, bass_isa
from concourse.bass_utils import run_bass_kernel_spmd
from concourse.masks import make_identity

B, N, M = 8, 8192, 8192
P = 128           # partitions / n-chunk size
MT = 512          # m tile (one PSUM bank of fp32)
G = 4             # m tiles per PSUM group
GF = G * MT       # 2048 free elems per group
NCH = N // P      # 64 n-chunks
NG = M // GF      # 4 groups of m
F32 = mybir.dt.float32
F32R = mybir.dt.float32r
F16 = mybir.dt.float16
BF16 = mybir.dt.bfloat16

_cache = {}


def build(precision="fp32r", repeat=1, opt=5):
    # opt>=1: tail uses transpose-merged tensor_reduce
    # opt==4: full-row staging [128,8192]: one TS (dist1 direct) + one TT
    #         per n-chunk; ~5% of casts shifted from ACT to DVE for balance
    # (opt 2/3 = GpSimd offload variants: rejected by this walrus, unused)
    if opt >= 6:
        return build_v2()
    key = (precision, repeat, opt)
    if key in _cache:
        return _cache[key]
    nc = bacc.Bacc()
    if precision == "fp32r":
        aug1 = nc.declare_dram_parameter("aug1", [5, N], F32, isOutput=False)
        aug2 = nc.declare_dram_parameter("aug2", [5, M], F32, isOutput=False)
    else:  # bf16x3: host-split hi/lo
        a1h = nc.declare_dram_parameter("a1h", [5, N], BF16, isOutput=False)
        a1l = nc.declare_dram_parameter("a1l", [5, N], BF16, isOutput=False)
        a2h = nc.declare_dram_parameter("a2h", [5, M], BF16, isOutput=False)
        a2l = nc.declare_dram_parameter("a2l", [5, M], BF16, isOutput=False)
    out1 = nc.declare_dram_parameter("out1", [P, NCH], F32, isOutput=True)
    out2 = nc.declare_dram_parameter("out2", [P, NCH], F32, isOutput=True)

    with tile.TileContext(nc) as tc:
        with tc.tile_pool(name="const", bufs=1) as const, \
             tc.tile_pool(name="stage", bufs=(2 if opt == 4 else 3)) as stage, \
             tc.tile_pool(name="small", bufs=4) as small, \
             tc.tile_pool(name="psum", bufs=2, space="PSUM") as psum:

            if precision == "fp32r":
                a1_sb = const.tile([5, N], F32)
                a2_sb = const.tile([5, M], F32)
                nc.sync.dma_start(out=a1_sb[:], in_=aug1[:])
                nc.sync.dma_start(out=a2_sb[:], in_=aug2[:])
                a1r = const.tile([5, N], F32R)
                a2r = const.tile([5, M], F32R)
                nc.vector.tensor_copy(a2r[:, 0:GF], a2_sb[:, 0:GF])
                nc.vector.tensor_copy(a1r[:, 0:P], a1_sb[:, 0:P])
                nc.vector.tensor_copy(a2r[:, GF:M], a2_sb[:, GF:M])
                nc.vector.tensor_copy(a1r[:, P:N], a1_sb[:, P:N])
                lhs_list = [a1r]
                rhs_list = [a2r]
            else:
                a1h_sb = const.tile([5, N], BF16)
                a1l_sb = const.tile([5, N], BF16)
                a2h_sb = const.tile([5, M], BF16)
                a2l_sb = const.tile([5, M], BF16)
                nc.sync.dma_start(out=a1h_sb[:], in_=a1h[:])
                nc.sync.dma_start(out=a1l_sb[:], in_=a1l[:])
                nc.sync.dma_start(out=a2h_sb[:], in_=a2h[:])
                nc.sync.dma_start(out=a2l_sb[:], in_=a2l[:])
                # hi*hi + hi*lo + lo*hi (lo*lo dropped, ~2^-16 relative)
                lhs_list = [a1h_sb, a1h_sb, a1l_sb]
                rhs_list = [a2h_sb, a2l_sb, a2h_sb]

            acc2 = const.tile([P, M], F16)
            nc.vector.memset(acc2[:], float("inf"))
            dist1_sb = const.tile([P, NCH], F32)
            dist2_sb = const.tile([P, NCH], F32)
            ident = const.tile([P, P], F16)
            make_identity(nc, ident[:])

            npass = len(lhs_list)
            for _rep in range(repeat):
              for nci in range(NCH):
                if opt >= 4:
                    # half-row (opt 5) or full-row (opt 4) staging
                    nh = 2 if opt >= 5 else 1
                    HF = M // nh
                    part1 = None
                    if nh > 1:
                        part1 = small.tile([P, nh], F32, tag="part1")
                    for h in range(nh):
                        sth = stage.tile([P, HF], F16, tag="st")
                        for g2 in range(NG // nh):
                            g = h * (NG // nh) + g2
                            ps = psum.tile([P, GF], F32, tag="ps")
                            for q in range(G):
                                mc = g * G + q
                                for ip in range(npass):
                                    nc.tensor.matmul(
                                        out=ps[:, q * MT:(q + 1) * MT],
                                        lhsT=lhs_list[ip][:, nci * P:(nci + 1) * P],
                                        rhs=rhs_list[ip][:, mc * MT:(mc + 1) * MT],
                                        start=(ip == 0),
                                        stop=(ip == npass - 1),
                                    )
                            if g == 1 and nci % 9 == 0:
                                nc.vector.tensor_copy(sth[:, g2 * GF:(g2 + 1) * GF], ps[:])
                            else:
                                nc.scalar.copy(out=sth[:, g2 * GF:(g2 + 1) * GF], in_=ps[:])
                        acc1_dst = dist1_sb[:, nci:nci + 1] if nh == 1 else part1[:, h:h + 1]
                        nc.vector.tensor_scalar(
                            out=sth[:], in0=sth[:], scalar1=0.0, scalar2=None,
                            op0=mybir.AluOpType.max, op1=mybir.AluOpType.min,
                            accum_out=acc1_dst,
                        )
                        nc.vector.tensor_tensor(
                            out=acc2[:, h * HF:(h + 1) * HF],
                            in0=acc2[:, h * HF:(h + 1) * HF], in1=sth[:],
                            op=mybir.AluOpType.min,
                        )
                    if nh > 1:
                        nc.vector.tensor_reduce(
                            out=dist1_sb[:, nci:nci + 1], in_=part1[:],
                            axis=mybir.AxisListType.X, op=mybir.AluOpType.min,
                        )
                    continue
                part1 = small.tile([P, NG], F32)
                for g in range(NG):
                    ps = psum.tile([P, GF], F32, tag="ps")
                    for q in range(G):
                        mc = g * G + q
                        for ip in range(npass):
                            nc.tensor.matmul(
                                out=ps[:, q * MT:(q + 1) * MT],
                                lhsT=lhs_list[ip][:, nci * P:(nci + 1) * P],
                                rhs=rhs_list[ip][:, mc * MT:(mc + 1) * MT],
                                start=(ip == 0),
                                stop=(ip == npass - 1),
                            )
                    st = stage.tile([P, GF], F16, tag="st")
                    nc.scalar.copy(out=st[:], in_=ps[:])
                    nc.vector.tensor_scalar(
                        out=st[:], in0=st[:], scalar1=0.0, scalar2=None,
                        op0=mybir.AluOpType.max, op1=mybir.AluOpType.min,
                        accum_out=part1[:, g:g + 1],
                    )
                    nc.vector.tensor_tensor(
                        out=acc2[:, g * GF:(g + 1) * GF],
                        in0=acc2[:, g * GF:(g + 1) * GF],
                        in1=st[:], op=mybir.AluOpType.min,
                    )
                nc.vector.tensor_reduce(
                    out=dist1_sb[:, nci:nci + 1], in_=part1[:],
                    axis=mybir.AxisListType.X, op=mybir.AluOpType.min,
                )

              # dist2 tail: transpose acc2 128x128 blocks, min-reduce over n
              if opt >= 1:
                for t4 in range(NCH // 4):
                    pst = psum.tile([P, 4, P], F16, tag="ps")
                    for q in range(4):
                        nc.tensor.transpose(
                            out=pst[:, q, :],
                            in_=acc2[:, (t4 * 4 + q) * P:(t4 * 4 + q + 1) * P],
                            identity=ident[:],
                        )
                    nc.vector.tensor_reduce(
                        out=dist2_sb[:, t4 * 4:t4 * 4 + 4], in_=pst[:],
                        axis=mybir.AxisListType.X, op=mybir.AluOpType.min,
                    )
                # final relu clamp for dist2 (dist1 clamps in the TS)
                nc.vector.tensor_scalar(
                    out=dist2_sb[:], in0=dist2_sb[:], scalar1=0.0, scalar2=None,
                    op0=mybir.AluOpType.max,
                )
              else:
                for t in range(NCH):
                    pst = psum.tile([P, P], F16, tag="ps")
                    nc.tensor.transpose(
                        out=pst[:], in_=acc2[:, t * P:(t + 1) * P], identity=ident[:]
                    )
                    scr = small.tile([P, P], F16)
                    nc.vector.tensor_scalar(
                        out=scr[:], in0=pst[:], scalar1=0.0, scalar2=None,
                        op0=mybir.AluOpType.max, op1=mybir.AluOpType.min,
                        accum_out=dist2_sb[:, t:t + 1],
                    )

            nc.sync.dma_start(out=out1[:], in_=dist1_sb[:])
            nc.sync.dma_start(out=out2[:], in_=dist2_sb[:])

    nc.finalize()
    _cache[key] = nc
    return nc


def build_v2(q=4, stage_bufs=3, hf2=None):
    """Negated-max formulation, no transpose tail.

    aug2n is pre-negated on the host so the matmul produces s = -d.
    Per chunk: ACT casts PSUM->st fp16 (3 or 4 of 4 groups); on every q-th
    chunk the DVE drains the last group via a fused TS (cast + clamp +
    max-accum -> dist1 partial).  One full-span TS (4x mode) gives the dist1
    partial of the ACT-cast span; one TT max merges st into acc2.  Tail:
    GpSimd partition_all_reduce(max) over acc2 quarters (in-place), DMA row 0.
    dist1/dist2 are negated+relu'd on the host.
    """
    key = ("v2", q, stage_bufs, hf2)
    if key in _cache:
        return _cache[key]
    HF2 = hf2 if hf2 is not None else GF // 2  # DVE partial-group drain size
    nc = bacc.Bacc()
    aug1 = nc.declare_dram_parameter("aug1", [5, N], F32R, isOutput=False)
    aug2n = nc.declare_dram_parameter("aug2n", [5, M], F32R, isOutput=False)
    out1 = nc.declare_dram_parameter("out1", [P, NCH], F32, isOutput=True)
    # dist2 leaves by two routes: first half of m via Pool all-reduce rows,
    # second half via the transpose tail
    out2a = nc.declare_dram_parameter("out2a", [1, M // 2], F16, isOutput=True)
    out2b = nc.declare_dram_parameter("out2b", [P, NCH // 2], F32, isOutput=True)

    with tile.TileContext(nc) as tc:
        with tc.tile_pool(name="const", bufs=1) as const, \
             tc.tile_pool(name="stage", bufs=stage_bufs) as stage, \
             tc.tile_pool(name="psum", bufs=2, space="PSUM") as psum:

            a1r = const.tile([5, N], F32R)
            a2r = const.tile([5, M], F32R)
            # first chunk needs a2r group 0 + a1r[0:P] first
            nc.sync.dma_start(out=a2r[:, 0:GF], in_=aug2n[:, 0:GF])
            nc.sync.dma_start(out=a1r[:, 0:P], in_=aug1[:, 0:P])
            nc.sync.dma_start(out=a2r[:, GF:M], in_=aug2n[:, GF:M])
            nc.sync.dma_start(out=a1r[:, P:N], in_=aug1[:, P:N])

            # PE p-state warm-up: dummy matmuls on a memset tile keep the PE
            # continuously busy through the input DMA latency so the first
            # real matmuls run at full clock
            warm = const.tile([5, MT], F32R)
            nc.vector.memset(warm[:].bitcast(F32), 0.0)
            wps = psum.tile([P, MT], F32, tag="ps")
            for _ in range(8):
                nc.tensor.matmul(out=wps[:], lhsT=warm[:, 0:P], rhs=warm[:],
                                 start=True, stop=True)

            acc2 = const.tile([P, M], F16)
            nc.gpsimd.memset(acc2[:], float("-inf"))
            parts0 = const.tile([P, NCH], F32)
            parts1 = const.tile([P, NCH], F32)
            nc.gpsimd.memset(parts1[:], -1e30)
            partsA = const.tile([P, NG], F32)
            partsB = const.tile([P, NG], F32)
            d1row = const.tile([P, NCH], F32)
            dist2_sb = const.tile([P, NCH // 2], F32)
            ident = const.tile([P, P], F16)
            make_identity(nc, ident[:])

            prev_st = None  # TT merge of chunk c is emitted during chunk c+1
            for nci in range(NCH):
                tailchunk = nci >= NCH - 2  # per-group processing
                last = nci == NCH - 1
                # on every q-th chunk DVE drains the top of the last group
                # (fused cast+clamp+dist1); ACT casts the lower part so it
                # stays busy through the PE refill latency
                dve_drain = q > 0 and (nci % q == q - 1) and not tailchunk
                st = stage.tile([P, M], F16, tag="st")
                if tailchunk and prev_st is not None:
                    nc.vector.tensor_tensor(
                        out=acc2[:], in0=acc2[:], in1=prev_st[:],
                        op=mybir.AluOpType.max,
                    )
                    prev_st = None
                for g in range(NG):
                    ps = psum.tile([P, GF], F32, tag="ps")
                    for mt in range(G):
                        mc = g * G + mt
                        nc.tensor.matmul(
                            out=ps[:, mt * MT:(mt + 1) * MT],
                            lhsT=a1r[:, nci * P:(nci + 1) * P],
                            rhs=a2r[:, mc * MT:(mc + 1) * MT],
                            start=True, stop=True,
                        )
                    if dve_drain and g == NG - 1:
                        nc.scalar.copy(
                            out=st[:, g * GF:g * GF + HF2], in_=ps[:, 0:HF2])
                        drain_ps = ps  # dTS emitted after the deferred TT below
                    elif tailchunk:
                        # pipelined tail: per-group cast, dist1 part, merge;
                        # on the very last chunk also partition all-reduce
                        # (Pool) + store, per half-group
                        sl = slice(g * GF, (g + 1) * GF)
                        nc.scalar.copy(out=st[:, sl], in_=ps[:])
                        nc.vector.tensor_scalar(
                            out=st[:, sl], in0=st[:, sl],
                            scalar1=0.0, scalar2=None,
                            op0=mybir.AluOpType.min, op1=mybir.AluOpType.max,
                            accum_out=(partsA if last else partsB)[:, g:g + 1],
                        )
                        nc.vector.tensor_tensor(
                            out=acc2[:, sl], in0=acc2[:, sl], in1=st[:, sl],
                            op=mybir.AluOpType.max,
                        )
                        if last and g < NG // 2:
                            for h in range(2):
                                slh = slice(g * GF + h * (GF // 2),
                                            g * GF + (h + 1) * (GF // 2))
                                nc.gpsimd.partition_all_reduce(
                                    acc2[:, slh], acc2[:, slh], channels=P,
                                    reduce_op=bass_isa.ReduceOp.max,
                                )
                                nc.sync.dma_start(
                                    out=out2a[:, slh], in_=acc2[0:1, slh])

                    else:
                        nc.scalar.copy(out=st[:, g * GF:(g + 1) * GF], in_=ps[:])
                if tailchunk:
                    continue
                if prev_st is not None:
                    nc.vector.tensor_tensor(
                        out=acc2[:], in0=acc2[:], in1=prev_st[:],
                        op=mybir.AluOpType.max,
                    )
                if dve_drain:
                    nc.vector.tensor_scalar(
                        out=st[:, NG * GF - (GF - HF2):NG * GF],
                        in0=drain_ps[:, HF2:GF],
                        scalar1=0.0, scalar2=None,
                        op0=mybir.AluOpType.min, op1=mybir.AluOpType.max,
                        accum_out=parts1[:, nci:nci + 1],
                    )
                span = NG * GF - (GF - HF2) if dve_drain else NG * GF
                nc.vector.tensor_scalar(
                    out=st[:, 0:span], in0=st[:, 0:span],
                    scalar1=0.0, scalar2=None,
                    op0=mybir.AluOpType.min, op1=mybir.AluOpType.max,
                    accum_out=parts0[:, nci:nci + 1],
                )
                prev_st = st

            # transpose tail for the second half of m: PE transposes finalized
            # acc2 blocks into PSUM (ring free), DVE TR-maxes the 128 n-lanes
            for t4 in range(NCH // 8, NCH // 4):
                pst = psum.tile([P, 4, P], F16, tag="ps")
                for qq in range(4):
                    blk = t4 * 4 + qq
                    nc.tensor.transpose(
                        out=pst[:, qq, :],
                        in_=acc2[:, blk * P:(blk + 1) * P],
                        identity=ident[:],
                    )
                col = t4 * 4 - NCH // 2
                nc.vector.tensor_reduce(
                    out=dist2_sb[:, col:col + 4],
                    in_=pst[:], axis=mybir.AxisListType.X,
                    op=mybir.AluOpType.max,
                )

            # dist1 parts of the two per-group tail chunks
            nc.vector.tensor_reduce(
                out=parts0[:, NCH - 2:NCH - 1], in_=partsB[:],
                axis=mybir.AxisListType.X, op=mybir.AluOpType.max,
            )
            nc.vector.tensor_reduce(
                out=parts0[:, NCH - 1:NCH], in_=partsA[:],
                axis=mybir.AxisListType.X, op=mybir.AluOpType.max,
            )
            nc.vector.tensor_tensor(
                out=d1row[:], in0=parts0[:], in1=parts1[:],
                op=mybir.AluOpType.max,
            )
            nc.sync.dma_start(out=out1[:], in_=d1row[:])
            nc.sync.dma_start(out=out2b[:], in_=dist2_sb[:])

    nc.finalize()
    _cache[key] = nc
    return nc


def _augment(p1, p2):
    """p1 [N,3], p2 [M,3] -> aug1 [5,N], aug2 [5,M] fp32 (C-contiguous)."""
    sq1 = (p1 * p1).sum(1)
    sq2 = (p2 * p2).sum(1)
    aug1 = np.empty((5, p1.shape[0]), np.float32)
    aug1[0:3] = p1.T
    aug1[3] = sq1
    aug1[4] = 1.0
    aug2 = np.empty((5, p2.shape[0]), np.float32)
    aug2[0:3] = -2.0 * p2.T
    aug2[3] = 1.0
    aug2[4] = sq2
    return aug1, aug2


def make_in_maps(input1, input2, precision="fp32r"):
    import ml_dtypes
    in_maps = []
    for b in range(B):
        p1 = np.ascontiguousarray(np.asarray(input1[b], np.float32))
        p2 = np.ascontiguousarray(np.asarray(input2[b], np.float32))
        aug1, aug2 = _augment(p1, p2)
        if precision == "fp32r":
            in_maps.append({"aug1": aug1, "aug2": aug2})
        else:
            a1h = aug1.astype(ml_dtypes.bfloat16)
            a1l = (aug1 - a1h.astype(np.float32)).astype(ml_dtypes.bfloat16)
            a2h = aug2.astype(ml_dtypes.bfloat16)
            a2l = (aug2 - a2h.astype(np.float32)).astype(ml_dtypes.bfloat16)
            in_maps.append({"a1h": a1h, "a1l": a1l, "a2h": a2h, "a2l": a2l})
    return in_maps


def make_in_maps_v2(input1, input2):
    in_maps = []
    for b in range(B):
        p1 = np.ascontiguousarray(np.asarray(input1[b], np.float32))
        p2 = np.ascontiguousarray(np.asarray(input2[b], np.float32))
        aug1, aug2 = _augment(p1, p2)
        in_maps.append({"aug1": aug1, "aug2n": np.ascontiguousarray(-aug2)})
    return in_maps


def run_v2(input1, input2, q=4, stage_bufs=3, trace=False):
    nc = build_v2(q=q, stage_bufs=stage_bufs)
    in_maps = make_in_maps_v2(input1, input2)
    res = run_bass_kernel_spmd(nc, in_maps, list(range(B)), trace=trace)
    # out1 [P, NCH]: lane p, chunk c -> n = c*128+p; value = max_m(-d) (clamped)
    dist1 = np.stack([
        np.maximum(-res.results[b]["out1"].T.reshape(N), 0.0) for b in range(B)
    ])
    dist2 = np.stack([
        np.concatenate([
            np.maximum(-np.asarray(res.results[b]["out2a"], np.float32).reshape(M // 2), 0.0),
            np.maximum(-res.results[b]["out2b"].T.reshape(M // 2), 0.0),
        ]) for b in range(B)
    ])
    return (dist1.astype(np.float32), dist2.astype(np.float32)), res


def run(input1, input2, precision="fp32r", trace=False, opt=5, repeat=1):
    if opt >= 6:
        return run_v2(input1, input2, trace=trace)
    nc = build(precision, repeat=repeat, opt=opt)
    in_maps = make_in_maps(input1, input2, precision)
    res = run_bass_kernel_spmd(nc, in_maps, list(range(B)), trace=trace)
    dist1 = np.stack([res.results[b]["out1"].T.reshape(N) for b in range(B)])
    dist2 = np.stack([res.results[b]["out2"].T.reshape(N) for b in range(B)])
    return (dist1.astype(np.float32), dist2.astype(np.float32)), res


def kernel(input1, input2):
    (dist1, dist2), _ = run(input1, input2, precision="fp32r", trace=False, opt=6)
    return (dist1, dist2)



# revision 36
# speedup vs baseline: 1.0785x; 1.0391x over previous
"""Chamfer distance kernel for Trainium2 (8 NeuronCores, data-parallel over batch).

Math: d[n,m] = |a_n|^2 + |b_m|^2 - 2 a_n . b_m, computed as a K=5 augmented
matmul: aug1 = [x,y,z,|a|^2,1], aug2 = [-2x,-2y,-2z,1,|b|^2], so
aug1[:,n] . aug2[:,m] = d[n,m].  dist1 = min over m (after relu clamp),
dist2 = min over n.

Per core (one batch):
  - PE: fp32r matmuls [5,128]x[5,512] -> PSUM fp32, grouped 4 banks at a time
  - ACT: cast PSUM fp32 [128,2048] -> SBUF fp16 (staged)
  - DVE tensor_scalar (4x mode): relu clamp in place + min-reduce over free
    -> dist1 partials
  - DVE tensor_tensor min: accumulate staged into acc2 [128, M] fp16 (dist2
    partial per m over the 128-lane n-residual)
  - tail: PE transpose acc2 in 128x128 blocks -> PSUM, TS min-reduce -> dist2
"""

import sys

sys.path.insert(0, "/opt/trn_rl_repo")

import numpy as np

import concourse.bass as bass  # noqa: F401  (bass types used via bacc/tile)
import concourse.mybir as mybir
import concourse.tile as tile
from concourse import bacc, bass_isa
from concourse.bass_utils import run_bass_kernel_spmd
from concourse.masks import make_identity

B, N, M = 8, 8192, 8192
P = 128           # partitions / n-chunk size
MT = 512          # m tile (one PSUM bank of fp32)
G = 4             # m tiles per PSUM group
GF = G * MT       # 2048 free elems per group
NCH = N // P      # 64 n-chunks
NG = M // GF      # 4 groups of m
F32 = mybir.dt.float32
F32R = mybir.dt.float32r
F16 = mybir.dt.float16
BF16 = mybir.dt.bfloat16

_cache = {}


def build(precision="fp32r", repeat=1, opt=5):
    # opt>=1: tail uses transpose-merged tensor_reduce
    # opt==4: full-row staging [128,8192]: one TS (dist1 direct) + one TT
    #         per n-chunk; ~5% of casts shifted from ACT to DVE for balance
    # (opt 2/3 = GpSimd offload variants: rejected by this walrus, unused)
    if opt >= 6:
        return build_v2()
    key = (precision, repeat, opt)
    if key in _cache:
        return _cache[key]
    nc = bacc.Bacc()
    if precision == "fp32r":
        aug1 = nc.declare_dram_parameter("aug1", [5, N], F32, isOutput=False)
        aug2 = nc.declare_dram_parameter("aug2", [5, M], F32, isOutput=False)
    else:  # bf16x3: host-split hi/lo
        a1h = nc.declare_dram_parameter("a1h", [5, N], BF16, isOutput=False)
        a1l = nc.declare_dram_parameter("a1l", [5, N], BF16, isOutput=False)
        a2h = nc.declare_dram_parameter("a2h", [5, M], BF16, isOutput=False)
        a2l = nc.declare_dram_parameter("a2l", [5, M], BF16, isOutput=False)
    out1 = nc.declare_dram_parameter("out1", [P, NCH], F32, isOutput=True)
    out2 = nc.declare_dram_parameter("out2", [P, NCH], F32, isOutput=True)

    with tile.TileContext(nc) as tc:
        with tc.tile_pool(name="const", bufs=1) as const, \
             tc.tile_pool(name="stage", bufs=(2 if opt == 4 else 3)) as stage, \
             tc.tile_pool(name="small", bufs=4) as small, \
             tc.tile_pool(name="psum", bufs=2, space="PSUM") as psum:

            if precision == "fp32r":
                a1_sb = const.tile([5, N], F32)
                a2_sb = const.tile([5, M], F32)
                nc.sync.dma_start(out=a1_sb[:], in_=aug1[:])
                nc.sync.dma_start(out=a2_sb[:], in_=aug2[:])
                a1r = const.tile([5, N], F32R)
                a2r = const.tile([5, M], F32R)
                nc.vector.tensor_copy(a2r[:, 0:GF], a2_sb[:, 0:GF])
                nc.vector.tensor_copy(a1r[:, 0:P], a1_sb[:, 0:P])
                nc.vector.tensor_copy(a2r[:, GF:M], a2_sb[:, GF:M])
                nc.vector.tensor_copy(a1r[:, P:N], a1_sb[:, P:N])
                lhs_list = [a1r]
                rhs_list = [a2r]
            else:
                a1h_sb = const.tile([5, N], BF16)
                a1l_sb = const.tile([5, N], BF16)
                a2h_sb = const.tile([5, M], BF16)
                a2l_sb = const.tile([5, M], BF16)
                nc.sync.dma_start(out=a1h_sb[:], in_=a1h[:])
                nc.sync.dma_start(out=a1l_sb[:], in_=a1l[:])
                nc.sync.dma_start(out=a2h_sb[:], in_=a2h[:])
                nc.sync.dma_start(out=a2l_sb[:], in_=a2l[:])
                # hi*hi + hi*lo + lo*hi (lo*lo dropped, ~2^-16 relative)
                lhs_list = [a1h_sb, a1h_sb, a1l_sb]
                rhs_list = [a2h_sb, a2l_sb, a2h_sb]

            acc2 = const.tile([P, M], F16)
            nc.vector.memset(acc2[:], float("inf"))
            dist1_sb = const.tile([P, NCH], F32)
            dist2_sb = const.tile([P, NCH], F32)
            ident = const.tile([P, P], F16)
            make_identity(nc, ident[:])

            npass = len(lhs_list)
            for _rep in range(repeat):
              for nci in range(NCH):
                if opt >= 4:
                    # half-row (opt 5) or full-row (opt 4) staging
                    nh = 2 if opt >= 5 else 1
                    HF = M // nh
                    part1 = None
                    if nh > 1:
                        part1 = small.tile([P, nh], F32, tag="part1")
                    for h in range(nh):
                        sth = stage.tile([P, HF], F16, tag="st")
                        for g2 in range(NG // nh):
                            g = h * (NG // nh) + g2
                            ps = psum.tile([P, GF], F32, tag="ps")
                            for q in range(G):
                                mc = g * G + q
                                for ip in range(npass):
                                    nc.tensor.matmul(
                                        out=ps[:, q * MT:(q + 1) * MT],
                                        lhsT=lhs_list[ip][:, nci * P:(nci + 1) * P],
                                        rhs=rhs_list[ip][:, mc * MT:(mc + 1) * MT],
                                        start=(ip == 0),
                                        stop=(ip == npass - 1),
                                    )
                            if g == 1 and nci % 9 == 0:
                                nc.vector.tensor_copy(sth[:, g2 * GF:(g2 + 1) * GF], ps[:])
                            else:
                                nc.scalar.copy(out=sth[:, g2 * GF:(g2 + 1) * GF], in_=ps[:])
                        acc1_dst = dist1_sb[:, nci:nci + 1] if nh == 1 else part1[:, h:h + 1]
                        nc.vector.tensor_scalar(
                            out=sth[:], in0=sth[:], scalar1=0.0, scalar2=None,
                            op0=mybir.AluOpType.max, op1=mybir.AluOpType.min,
                            accum_out=acc1_dst,
                        )
                        nc.vector.tensor_tensor(
                            out=acc2[:, h * HF:(h + 1) * HF],
                            in0=acc2[:, h * HF:(h + 1) * HF], in1=sth[:],
                            op=mybir.AluOpType.min,
                        )
                    if nh > 1:
                        nc.vector.tensor_reduce(
                            out=dist1_sb[:, nci:nci + 1], in_=part1[:],
                            axis=mybir.AxisListType.X, op=mybir.AluOpType.min,
                        )
                    continue
                part1 = small.tile([P, NG], F32)
                for g in range(NG):
                    ps = psum.tile([P, GF], F32, tag="ps")
                    for q in range(G):
                        mc = g * G + q
                        for ip in range(npass):
                            nc.tensor.matmul(
                                out=ps[:, q * MT:(q + 1) * MT],
                                lhsT=lhs_list[ip][:, nci * P:(nci + 1) * P],
                                rhs=rhs_list[ip][:, mc * MT:(mc + 1) * MT],
                                start=(ip == 0),
                                stop=(ip == npass - 1),
                            )
                    st = stage.tile([P, GF], F16, tag="st")
                    nc.scalar.copy(out=st[:], in_=ps[:])
                    nc.vector.tensor_scalar(
                        out=st[:], in0=st[:], scalar1=0.0, scalar2=None,
                        op0=mybir.AluOpType.max, op1=mybir.AluOpType.min,
                        accum_out=part1[:, g:g + 1],
                    )
                    nc.vector.tensor_tensor(
                        out=acc2[:, g * GF:(g + 1) * GF],
                        in0=acc2[:, g * GF:(g + 1) * GF],
                        in1=st[:], op=mybir.AluOpType.min,
                    )
                nc.vector.tensor_reduce(
                    out=dist1_sb[:, nci:nci + 1], in_=part1[:],
                    axis=mybir.AxisListType.X, op=mybir.AluOpType.min,
                )

              # dist2 tail: transpose acc2 128x128 blocks, min-reduce over n
              if opt >= 1:
                for t4 in range(NCH // 4):
                    pst = psum.tile([P, 4, P], F16, tag="ps")
                    for q in range(4):
                        nc.tensor.transpose(
                            out=pst[:, q, :],
                            in_=acc2[:, (t4 * 4 + q) * P:(t4 * 4 + q + 1) * P],
                            identity=ident[:],
                        )
                    nc.vector.tensor_reduce(
                        out=dist2_sb[:, t4 * 4:t4 * 4 + 4], in_=pst[:],
                        axis=mybir.AxisListType.X, op=mybir.AluOpType.min,
                    )
                # final relu clamp for dist2 (dist1 clamps in the TS)
                nc.vector.tensor_scalar(
                    out=dist2_sb[:], in0=dist2_sb[:], scalar1=0.0, scalar2=None,
                    op0=mybir.AluOpType.max,
                )
              else:
                for t in range(NCH):
                    pst = psum.tile([P, P], F16, tag="ps")
                    nc.tensor.transpose(
                        out=pst[:], in_=acc2[:, t * P:(t + 1) * P], identity=ident[:]
                    )
                    scr = small.tile([P, P], F16)
                    nc.vector.tensor_scalar(
                        out=scr[:], in0=pst[:], scalar1=0.0, scalar2=None,
                        op0=mybir.AluOpType.max, op1=mybir.AluOpType.min,
                        accum_out=dist2_sb[:, t:t + 1],
                    )

            nc.sync.dma_start(out=out1[:], in_=dist1_sb[:])
            nc.sync.dma_start(out=out2[:], in_=dist2_sb[:])

    nc.finalize()
    _cache[key] = nc
    return nc


def build_v2(q=1, stage_bufs=3, hf2=1600):
    """Negated-max formulation, no transpose tail.

    aug2n is pre-negated on the host so the matmul produces s = -d.
    Per chunk: ACT casts PSUM->st fp16 (3 or 4 of 4 groups); on every q-th
    chunk the DVE drains the last group via a fused TS (cast + clamp +
    max-accum -> dist1 partial).  One full-span TS (4x mode) gives the dist1
    partial of the ACT-cast span; one TT max merges st into acc2.  Tail:
    GpSimd partition_all_reduce(max) over acc2 quarters (in-place), DMA row 0.
    dist1/dist2 are negated+relu'd on the host.
    """
    key = ("v2", q, stage_bufs, hf2)
    if key in _cache:
        return _cache[key]
    HF2 = hf2 if hf2 is not None else GF // 2  # DVE partial-group drain size
    nc = bacc.Bacc()
    aug1 = nc.declare_dram_parameter("aug1", [5, N], F32R, isOutput=False)
    aug2n = nc.declare_dram_parameter("aug2n", [5, M], F32R, isOutput=False)
    out1 = nc.declare_dram_parameter("out1", [P, NCH], F32, isOutput=True)
    # dist2 leaves by two routes: first half of m via Pool all-reduce rows,
    # second half via the transpose tail
    out2a = nc.declare_dram_parameter("out2a", [1, M // 2], F16, isOutput=True)
    out2b = nc.declare_dram_parameter("out2b", [P, NCH // 2], F32, isOutput=True)

    with tile.TileContext(nc) as tc:
        with tc.tile_pool(name="const", bufs=1) as const, \
             tc.tile_pool(name="stage", bufs=stage_bufs) as stage, \
             tc.tile_pool(name="psum", bufs=2, space="PSUM") as psum:

            a1r = const.tile([5, N], F32R)
            a2r = const.tile([5, M], F32R)
            # first chunk needs a2r group 0 + a1r[0:P] first
            nc.sync.dma_start(out=a2r[:, 0:GF], in_=aug2n[:, 0:GF])
            nc.sync.dma_start(out=a1r[:, 0:P], in_=aug1[:, 0:P])
            nc.sync.dma_start(out=a2r[:, GF:M], in_=aug2n[:, GF:M])
            nc.sync.dma_start(out=a1r[:, P:N], in_=aug1[:, P:N])

            # PE p-state warm-up: dummy matmuls on a memset tile keep the PE
            # continuously busy through the input DMA latency so the first
            # real matmuls run at full clock
            warm = const.tile([5, MT], F32R)
            nc.vector.memset(warm[:].bitcast(F32), 0.0)
            wps = psum.tile([P, MT], F32, tag="ps")
            for _ in range(8):
                nc.tensor.matmul(out=wps[:], lhsT=warm[:, 0:P], rhs=warm[:],
                                 start=True, stop=True)

            acc2 = const.tile([P, M], F16)
            nc.gpsimd.memset(acc2[:], float("-inf"))
            parts0 = const.tile([P, NCH], F32)
            parts1 = const.tile([P, NCH], F32)
            nc.gpsimd.memset(parts1[:], -1e30)
            partsA = const.tile([P, NG], F32)
            partsB = const.tile([P, NG], F32)
            d1row = const.tile([P, NCH], F32)
            dist2_sb = const.tile([P, NCH // 2], F32)
            ident = const.tile([P, P], F16)
            make_identity(nc, ident[:])

            prev_st = None  # TT merge of chunk c is emitted during chunk c+1
            for nci in range(NCH):
                tailchunk = nci >= NCH - 2  # per-group processing
                last = nci == NCH - 1
                # on every q-th chunk DVE drains the top of the last group
                # (fused cast+clamp+dist1); ACT casts the lower part so it
                # stays busy through the PE refill latency
                dve_drain = q > 0 and (nci % q == q - 1) and not tailchunk
                st = stage.tile([P, M], F16, tag="st")
                if tailchunk and prev_st is not None:
                    nc.vector.tensor_tensor(
                        out=acc2[:], in0=acc2[:], in1=prev_st[:],
                        op=mybir.AluOpType.max,
                    )
                    prev_st = None
                for g in range(NG):
                    ps = psum.tile([P, GF], F32, tag="ps")
                    for mt in range(G):
                        mc = g * G + mt
                        nc.tensor.matmul(
                            out=ps[:, mt * MT:(mt + 1) * MT],
                            lhsT=a1r[:, nci * P:(nci + 1) * P],
                            rhs=a2r[:, mc * MT:(mc + 1) * MT],
                            start=True, stop=True,
                        )
                    if dve_drain and g == NG - 1:
                        nc.scalar.copy(
                            out=st[:, g * GF:g * GF + HF2], in_=ps[:, 0:HF2])
                        drain_ps = ps  # dTS emitted after the deferred TT below
                    elif tailchunk:
                        # pipelined tail: per-group cast, dist1 part, merge;
                        # on the very last chunk also partition all-reduce
                        # (Pool) + store, per half-group
                        sl = slice(g * GF, (g + 1) * GF)
                        nc.scalar.copy(out=st[:, sl], in_=ps[:])
                        nc.vector.tensor_scalar(
                            out=st[:, sl], in0=st[:, sl],
                            scalar1=0.0, scalar2=None,
                            op0=mybir.AluOpType.min, op1=mybir.AluOpType.max,
                            accum_out=(partsA if last else partsB)[:, g:g + 1],
                        )
                        nc.vector.tensor_tensor(
                            out=acc2[:, sl], in0=acc2[:, sl], in1=st[:, sl],
                            op=mybir.AluOpType.max,
                        )
                        if last and g < NG // 2:
                            for h in range(2):
                                slh = slice(g * GF + h * (GF // 2),
                                            g * GF + (h + 1) * (GF // 2))
                                nc.gpsimd.partition_all_reduce(
                                    acc2[:, slh], acc2[:, slh], channels=P,
                                    reduce_op=bass_isa.ReduceOp.max,
                                )
                                nc.sync.dma_start(
                                    out=out2a[:, slh], in_=acc2[0:1, slh])

                    else:
                        nc.scalar.copy(out=st[:, g * GF:(g + 1) * GF], in_=ps[:])
                if tailchunk:
                    continue
                if prev_st is not None:
                    nc.vector.tensor_tensor(
                        out=acc2[:], in0=acc2[:], in1=prev_st[:],
                        op=mybir.AluOpType.max,
                    )
                if dve_drain:
                    nc.vector.tensor_scalar(
                        out=st[:, NG * GF - (GF - HF2):NG * GF],
                        in0=drain_ps[:, HF2:GF],
                        scalar1=0.0, scalar2=None,
                        op0=mybir.AluOpType.min, op1=mybir.AluOpType.max,
                        accum_out=parts1[:, nci:nci + 1],
                    )
                span = NG * GF - (GF - HF2) if dve_drain else NG * GF
                nc.vector.tensor_scalar(
                    out=st[:, 0:span], in0=st[:, 0:span],
                    scalar1=0.0, scalar2=None,
                    op0=mybir.AluOpType.min, op1=mybir.AluOpType.max,
                    accum_out=parts0[:, nci:nci + 1],
                )
                prev_st = st

            # transpose tail for the second half of m: PE transposes finalized
            # acc2 blocks into PSUM (ring free), DVE TR-maxes the 128 n-lanes
            for t4 in range(NCH // 8, NCH // 4):
                pst = psum.tile([P, 4, P], F16, tag="ps")
                for qq in range(4):
                    blk = t4 * 4 + qq
                    nc.tensor.transpose(
                        out=pst[:, qq, :],
                        in_=acc2[:, blk * P:(blk + 1) * P],
                        identity=ident[:],
                    )
                col = t4 * 4 - NCH // 2
                nc.vector.tensor_reduce(
                    out=dist2_sb[:, col:col + 4],
                    in_=pst[:], axis=mybir.AxisListType.X,
                    op=mybir.AluOpType.max,
                )

            # dist1 parts of the two per-group tail chunks
            nc.vector.tensor_reduce(
                out=parts0[:, NCH - 2:NCH - 1], in_=partsB[:],
                axis=mybir.AxisListType.X, op=mybir.AluOpType.max,
            )
            nc.vector.tensor_reduce(
                out=parts0[:, NCH - 1:NCH], in_=partsA[:],
                axis=mybir.AxisListType.X, op=mybir.AluOpType.max,
            )
            nc.vector.tensor_tensor(
                out=d1row[:], in0=parts0[:], in1=parts1[:],
                op=mybir.AluOpType.max,
            )
            nc.sync.dma_start(out=out1[:], in_=d1row[:])
            nc.sync.dma_start(out=out2b[:], in_=dist2_sb[:])

    nc.finalize()
    _cache[key] = nc
    return nc


def _augment(p1, p2):
    """p1 [N,3], p2 [M,3] -> aug1 [5,N], aug2 [5,M] fp32 (C-contiguous)."""
    sq1 = (p1 * p1).sum(1)
    sq2 = (p2 * p2).sum(1)
    aug1 = np.empty((5, p1.shape[0]), np.float32)
    aug1[0:3] = p1.T
    aug1[3] = sq1
    aug1[4] = 1.0
    aug2 = np.empty((5, p2.shape[0]), np.float32)
    aug2[0:3] = -2.0 * p2.T
    aug2[3] = 1.0
    aug2[4] = sq2
    return aug1, aug2


def make_in_maps(input1, input2, precision="fp32r"):
    import ml_dtypes
    in_maps = []
    for b in range(B):
        p1 = np.ascontiguousarray(np.asarray(input1[b], np.float32))
        p2 = np.ascontiguousarray(np.asarray(input2[b], np.float32))
        aug1, aug2 = _augment(p1, p2)
        if precision == "fp32r":
            in_maps.append({"aug1": aug1, "aug2": aug2})
        else:
            a1h = aug1.astype(ml_dtypes.bfloat16)
            a1l = (aug1 - a1h.astype(np.float32)).astype(ml_dtypes.bfloat16)
            a2h = aug2.astype(ml_dtypes.bfloat16)
            a2l = (aug2 - a2h.astype(np.float32)).astype(ml_dtypes.bfloat16)
            in_maps.append({"a1h": a1h, "a1l": a1l, "a2h": a2h, "a2l": a2l})
    return in_maps


def make_in_maps_v2(input1, input2):
    in_maps = []
    for b in range(B):
        p1 = np.ascontiguousarray(np.asarray(input1[b], np.float32))
        p2 = np.ascontiguousarray(np.asarray(input2[b], np.float32))
        aug1, aug2 = _augment(p1, p2)
        in_maps.append({"aug1": aug1, "aug2n": np.ascontiguousarray(-aug2)})
    return in_maps


def run_v2(input1, input2, q=1, stage_bufs=3, hf2=1600, trace=False):
    nc = build_v2(q=q, stage_bufs=stage_bufs, hf2=hf2)
    in_maps = make_in_maps_v2(input1, input2)
    res = run_bass_kernel_spmd(nc, in_maps, list(range(B)), trace=trace)
    # out1 [P, NCH]: lane p, chunk c -> n = c*128+p; value = max_m(-d) (clamped)
    dist1 = np.stack([
        np.maximum(-res.results[b]["out1"].T.reshape(N), 0.0) for b in range(B)
    ])
    dist2 = np.stack([
        np.concatenate([
            np.maximum(-np.asarray(res.results[b]["out2a"], np.float32).reshape(M // 2), 0.0),
            np.maximum(-res.results[b]["out2b"].T.reshape(M // 2), 0.0),
        ]) for b in range(B)
    ])
    return (dist1.astype(np.float32), dist2.astype(np.float32)), res


def run(input1, input2, precision="fp32r", trace=False, opt=5, repeat=1):
    if opt >= 6:
        return run_v2(input1, input2, trace=trace)
    nc = build(precision, repeat=repeat, opt=opt)
    in_maps = make_in_maps(input1, input2, precision)
    res = run_bass_kernel_spmd(nc, in_maps, list(range(B)), trace=trace)
    dist1 = np.stack([res.results[b]["out1"].T.reshape(N) for b in range(B)])
    dist2 = np.stack([res.results[b]["out2"].T.reshape(N) for b in range(B)])
    return (dist1.astype(np.float32), dist2.astype(np.float32)), res


def kernel(input1, input2):
    (dist1, dist2), _ = run(input1, input2, precision="fp32r", trace=False, opt=6)
    return (dist1, dist2)

